# revision 11
# baseline (speedup 1.0000x reference)
"""Trainium2 Bass kernel for nn_Coembedding (dual-MLP cosine-similarity retrieval).

Computation (see reference):
    mp = relu(molecule @ Wm1.T + bm1) @ Wm2.T + bm2          [N, D]
    pp = relu(protein  @ Wp1.T + bp1) @ Wp2.T + bp2          [M, D]
    out = (pp/|pp| @ (mp/|mp|).T) / temperature              [M, N]

Distribution over 8 NeuronCores:
  - molecule rows (N) sharded 8x for the molecule MLP.  The UNNORMALIZED
    bf16 embeddings stream to the AllGather send buffer per output chunk as
    layer 2 evicts them, so the collective starts as early as possible and
    drains behind the protein MLP.  |mp| column norms ship to the host as a
    tiny side output; the host divides the output columns (the similarity
    GEMM is scale-invariant per column, so this is exact).
  - protein rows (M) sharded 8x; each core computes its own protein MLP
    shard and the [M/8, N] similarity tile.  1/|pp|/temperature folds into
    the per-partition eviction scale of the similarity tiles.

All on-chip layouts are feature-major (K on partitions) so the two MLP layers
and the similarity GEMM chain without transposes.  All matmul operands are
bf16 (fp32 PSUM accumulate): halves HBM/DMA traffic vs fp32 and keeps the
tensor engine at full rate; measured end-to-end error ~6e-3 vs the 2e-2 gate.
ALL weights preload into SBUF during the first ~30us, before the collective's
ring DMA saturates HBM, so no compute-feeding DMA ever contends with the
AllGather.  Output tiles are stored bf16 (alternating DMA queues) and upcast
host-side.
"""

import numpy as np
from contextlib import ExitStack

import ml_dtypes

import concourse.bass as bass
import concourse.tile as tile
from concourse import bacc, mybir
from concourse.bass_utils import run_bass_kernel_spmd

F32 = mybir.dt.float32
F32R = mybir.dt.float32r
BF16 = mybir.dt.bfloat16
AF = mybir.ActivationFunctionType

N_CORES = 8
N, M, MOL, PROT, D = 4096, 8192, 768, 1280, 1024
NS = N // N_CORES            # 512 molecule rows per core
MS = M // N_CORES            # 1024 protein rows per core
KM, KP, KD = MOL // 128, PROT // 128, D // 128   # 6, 10, 8 contraction chunks
DC = D // 128                # 8 output-feature chunks
EPS = 1e-8

_CACHE: dict = {}


def _build():
    if "nc" in _CACHE:
        return _CACHE["nc"]

    nc = bacc.Bacc("TRN2", target_bir_lowering=False, debug=False,
                   num_devices=N_CORES)

    # All inputs pre-tiled host-side; every DMA below is partition-major linear.
    molT = nc.dram_tensor("molT", [128, KM, NS], BF16, kind="ExternalInput").ap()
    protT = nc.dram_tensor("protT", [128, KP, MS], BF16, kind="ExternalInput").ap()
    wm1 = nc.dram_tensor("wm1", [128, DC, KM * 128], BF16, kind="ExternalInput").ap()
    wm2 = nc.dram_tensor("wm2", [128, DC, KD * 128], BF16, kind="ExternalInput").ap()
    wp1 = nc.dram_tensor("wp1", [128, DC, KP * 128], BF16, kind="ExternalInput").ap()
    wp2 = nc.dram_tensor("wp2", [128, DC, KD * 128], BF16, kind="ExternalInput").ap()
    bm1 = nc.dram_tensor("bm1", [128, DC], F32, kind="ExternalInput").ap()
    bm2 = nc.dram_tensor("bm2", [128, DC], F32, kind="ExternalInput").ap()
    bp1 = nc.dram_tensor("bp1", [128, DC], F32, kind="ExternalInput").ap()
    bp2 = nc.dram_tensor("bp2", [128, DC], F32, kind="ExternalInput").ap()
    invtemp = nc.dram_tensor("invtemp", [1, 1], F32, kind="ExternalInput").ap()
    ones_d = nc.dram_tensor("ones", [128, 128], F32R, kind="ExternalInput").ap()
    # S[c] = my protein shard vs (unnormalized) molecule shard c, bf16.
    S = nc.dram_tensor("S", [N_CORES, DC, 128, NS], BF16, kind="ExternalOutput").ap()
    # |mp| for my molecule shard's columns (host divides columns by these).
    Mnorm = nc.dram_tensor("Mnorm", [1, NS], F32, kind="ExternalOutput").ap()

    with tile.TileContext(nc) as tc, ExitStack() as ctx, \
            nc.allow_low_precision(reason="bf16 operands, fp32 accumulate"):
        dram = ctx.enter_context(tc.tile_pool(name="dram", bufs=1, space="DRAM"))
        send = dram.tile([128, DC, NS], BF16)            # Mb shard, partition-major
        recv = dram.tile([N_CORES, 128, DC, NS], BF16, addr_space="Shared")

        sb = ctx.enter_context(tc.tile_pool(name="sb", bufs=1))
        mn_pool = ctx.enter_context(tc.tile_pool(name="mn", bufs=2))
        st_pool = ctx.enter_context(tc.tile_pool(name="st", bufs=4))
        ps = ctx.enter_context(tc.tile_pool(name="ps", bufs=4, space="PSUM"))
        psn = ctx.enter_context(tc.tile_pool(name="psn", bufs=2, space="PSUM"))
        psb = ctx.enter_context(tc.tile_pool(name="psb", bufs=2, space="PSUM"))

        # ---- upfront loads: inputs + ALL weights into SBUF ----
        # sync: wm1, wp1.  scalar: molT, wm2, wp2.  gpsimd: consts, protT.
        # Everything lands before the AllGather's ring DMA saturates HBM.
        molT_s = sb.tile([128, KM, NS], BF16, tag="molT")
        nc.scalar.dma_start(out=molT_s[:], in_=molT[:])

        def load_w(name, dram_ap, kchunks, engine):
            t = sb.tile([128, DC, kchunks, 128], BF16, tag=name)
            src = dram_ap.rearrange("p h (k m) -> p h k m", k=kchunks)
            engine.dma_start(out=t[:], in_=src)
            return t

        # wm1 feeds the very first matmuls: split it across all three queues
        # (early DMA rate is the cold-start limiter) so each h slab lands
        # just ahead of its matmul group.
        wm1_s = sb.tile([128, DC, KM, 128], BF16, tag="wm1")
        wm1_src = wm1.rearrange("p h (k m) -> p h k m", k=KM)
        nc.sync.dma_start(out=wm1_s[:, 0:1], in_=wm1_src[:, 0:1])
        nc.sync.dma_start(out=wm1_s[:, 1:4], in_=wm1_src[:, 1:4])
        nc.scalar.dma_start(out=wm1_s[:, 4:6], in_=wm1_src[:, 4:6])
        nc.gpsimd.dma_start(out=wm1_s[:, 6:8], in_=wm1_src[:, 6:8])
        wm2_s = load_w("wm2", wm2, KD, nc.scalar)
        wp1_s = load_w("wp1", wp1, KP, nc.sync)
        wp2_s = load_w("wp2", wp2, KD, nc.scalar)

        ones_col = sb.tile([128, 1], F32R, tag="ones_col")
        nc.gpsimd.dma_start(out=ones_col[:], in_=ones_d[:, 0:1])
        invt = sb.tile([128, 1], F32, tag="invt")
        nc.gpsimd.dma_start(out=invt[:], in_=invtemp.to_broadcast([128, 1]))

        def load_bias(name, ap):
            t = sb.tile([128, DC], F32, tag=name)
            nc.gpsimd.dma_start(out=t[:], in_=ap[:])
            return t

        bm1_s, bm2_s = load_bias("bm1", bm1), load_bias("bm2", bm2)
        bp1_s, bp2_s = load_bias("bp1", bp1), load_bias("bp2", bp2)

        protT_s = sb.tile([128, KP, MS], BF16, tag="protT")
        nc.gpsimd.dma_start(out=protT_s[:], in_=protT[:])

        def mlp_layer(x_tile, w_sbuf, kchunks, ncols, bias_tile, relu, out_tile,
                      tail_h=None):
            """out[128, DC, ncols] = act(w.T @ x + b); all feature-major."""
            nhalves = ncols // 512
            for h in range(DC):
                for nh in range(nhalves):
                    pt = ps.tile([128, 512], F32, tag="mm")
                    for k in range(kchunks):
                        nc.tensor.matmul(
                            pt[:], w_sbuf[:, h, k, :],
                            x_tile[:, k, nh * 512:(nh + 1) * 512],
                            start=(k == 0), stop=(k == kchunks - 1),
                        )
                    nc.scalar.activation(
                        out_tile[:, h, nh * 512:(nh + 1) * 512], pt[:],
                        AF.Relu if relu else AF.Identity,
                        bias=bias_tile[:, h:h + 1],
                    )
                if tail_h is not None:
                    tail_h(h)

        def norm_accum(pn, x_tile, h, lo, width):
            """Accumulate column |x|^2 sums for d-chunk h into PSUM tile pn
            (interleaved with the layer's own matmul groups)."""
            sq = st_pool.tile([128, width], F32R, tag="sq", bufs=8)
            nc.vector.tensor_mul(sq[:], x_tile[:, h, lo:lo + width],
                                 x_tile[:, h, lo:lo + width])
            nc.tensor.matmul(pn[:], ones_col[:], sq[:],
                             start=(h == 0), stop=(h == DC - 1),
                             skip_group_check=True)

        # ================= molecule MLP (N shard) =================
        Hm = sb.tile([128, DC, NS], BF16, tag="hid")
        mlp_layer(molT_s, wm1_s, KM, NS, bm1_s, True, Hm)
        # layer 2 evicts bf16, streams each chunk to the send buffer, and
        # folds the |mp|^2 column-sum accumulation into the loop
        Mb = sb.tile([128, DC, NS], BF16, tag="emb")
        pn_m = psn.tile([1, NS], F32, tag="psn")

        def mol_tail(h):
            nc.gpsimd.dma_start(out=send[:, h, :], in_=Mb[:, h, :])
            norm_accum(pn_m, Mb, h, 0, NS)

        mlp_layer(Hm, wm2_s, KD, NS, bm2_s, False, Mb, tail_h=mol_tail)

        # ================= AllGather molecule embeddings =================
        nc.gpsimd.collective_compute(
            "AllGather",
            mybir.AluOpType.bypass,
            replica_groups=[list(range(N_CORES))],
            ins=[send[:]],
            outs=[recv[:]],
        )

        # |mp| column norms -> host (off the collective's critical path)
        nsq_m = sb.tile([1, NS], F32, tag="normsq_m")
        nc.scalar.activation(nsq_m[:], pn_m[:], AF.Sqrt)
        nc.scalar.dma_start(out=Mnorm[:], in_=nsq_m[:])

        # ================= protein MLP (M shard) =================
        Hp = sb.tile([128, DC, MS], BF16, tag="hidp")
        mlp_layer(protT_s, wp1_s, KP, MS, bp1_s, True, Hp)
        # L2 evicts straight to bf16; |pp|^2 column sums accumulate in-loop
        Ppb = sb.tile([128, DC, MS], BF16, tag="ppb")
        pn_p0 = psn.tile([1, 512], F32, tag="psn")
        pn_p1 = psn.tile([1, 512], F32, tag="psn")

        def prot_tail(h):
            norm_accum(pn_p0, Ppb, h, 0, 512)
            norm_accum(pn_p1, Ppb, h, 512, 512)

        mlp_layer(Hp, wp2_s, KD, MS, bp2_s, False, Ppb, tail_h=prot_tail)
        nsq_p = sb.tile([1, MS], F32, tag="normsq_p")
        nc.scalar.activation(nsq_p[:, 0:512], pn_p0[:], AF.Sqrt)
        nc.scalar.activation(nsq_p[:, 512:], pn_p1[:], AF.Sqrt)
        # transpose [1, MS] -> column form [128, DC] FIRST (tensor only waits
        # on the sqrt), then max/recip/scale run on vector off the PE path.
        ones_f32 = sb.tile([1, 1], F32, tag="ones_f32")
        nc.scalar.activation(ones_f32[:], ones_col[0:1, 0:1], AF.Copy)
        pcol = psb.tile([128, DC], F32, tag="psb")
        for j in range(DC):
            nc.tensor.matmul(pcol[:, j:j + 1], nsq_p[0:1, j * 128:(j + 1) * 128],
                             ones_f32[0:1, 0:1], start=(j == 0), stop=(j == DC - 1))
        scale_col = sb.tile([128, DC], F32, tag="scale_col")
        nc.vector.tensor_scalar_max(scale_col[:], pcol[:], EPS)
        nc.vector.reciprocal(scale_col[:], scale_col[:])
        nc.vector.tensor_scalar_mul(scale_col[:], scale_col[:], invt[:, 0:1])

        # ================= similarity tiles =================
        for c in range(N_CORES):
            mnb = mn_pool.tile([128, DC, NS], BF16, tag="mn")
            nc.sync.dma_start(out=mnb[:], in_=recv[c])
            for mi in range(MS // 128):
                pt = ps.tile([128, 512], F32, tag="mm")
                for k in range(KD):
                    nc.tensor.matmul(
                        pt[:], Ppb[:, k, mi * 128:(mi + 1) * 128],
                        mnb[:, k, :],
                        start=(k == 0), stop=(k == KD - 1),
                    )
                stile = st_pool.tile([128, NS], BF16, tag="stile")
                nc.scalar.activation(stile[:], pt[:], AF.Copy,
                                     scale=scale_col[:, mi:mi + 1])
                # alternate queues so the store stream never backs up
                (nc.gpsimd if mi % 2 == 0 else nc.sync).dma_start(
                    out=S[c, mi], in_=stile[:])

    nc.compile()
    _CACHE["nc"] = nc
    return nc


def _tile_w(W):
    """W [D, K] (fp32) -> [128, DC, K] where element (p, h, k*128+m) =
    W[h*128+m, k*128+p]: whole tensor is one partition-major linear DMA."""
    Dout, K = W.shape
    kc = K // 128
    t = W.reshape(DC, 128, kc, 128).transpose(3, 0, 2, 1)   # [p, h, k, m]
    return np.ascontiguousarray(t.reshape(128, DC, kc * 128)).astype(
        ml_dtypes.bfloat16)


def _tile_x(Xshard):
    """X [rows, K] -> [128, KC, rows] feature-major partition-tiled."""
    rows, K = Xshard.shape
    kc = K // 128
    t = Xshard.reshape(rows, kc, 128).transpose(2, 1, 0)    # [p, k, rows]
    return np.ascontiguousarray(t).astype(ml_dtypes.bfloat16)


def kernel(molecule, protein, Wm1, bm1, Wm2, bm2, Wp1, bp1, Wp2, bp2,
           temperature):
    nc = _build()

    molecule = np.asarray(molecule, np.float32)
    protein = np.asarray(protein, np.float32)
    wm1 = _tile_w(np.asarray(Wm1, np.float32))
    wm2 = _tile_w(np.asarray(Wm2, np.float32))
    wp1 = _tile_w(np.asarray(Wp1, np.float32))
    wp2 = _tile_w(np.asarray(Wp2, np.float32))

    def tile_b(b):
        return np.ascontiguousarray(np.asarray(b, np.float32).reshape(DC, 128).T)

    bm1_np, bm2_np = tile_b(bm1), tile_b(bm2)
    bp1_np, bp2_np = tile_b(bp1), tile_b(bp2)
    invt = (1.0 / np.asarray(temperature, np.float32)).reshape(1, 1)
    ones_np = np.ones((128, 128), np.float32)

    in_maps = []
    for c in range(N_CORES):
        in_maps.append({
            "molT": _tile_x(molecule[c * NS:(c + 1) * NS]),
            "protT": _tile_x(protein[c * MS:(c + 1) * MS]),
            "wm1": wm1, "wm2": wm2, "wp1": wp1, "wp2": wp2,
            "bm1": bm1_np, "bm2": bm2_np, "bp1": bp1_np, "bp2": bp2_np,
            "invtemp": invt, "ones": ones_np,
        })

    _CACHE["in_maps"] = in_maps
    res = run_bass_kernel_spmd(nc, in_maps, list(range(N_CORES)))
    out = np.empty((M, N), np.float32)
    # molecule-column norms: shard c2 covers output columns c2*NS..(c2+1)*NS
    normfull = np.concatenate(
        [np.asarray(res.results[c]["Mnorm"], np.float32).ravel()
         for c in range(N_CORES)])
    inv_cols = (1.0 / np.maximum(normfull, EPS)).astype(np.float32)
    for c in range(N_CORES):
        # S block layout [c2, mi, 128, 512] -> rows mi*128+i, cols c2*512+j
        blk = np.asarray(res.results[c]["S"], dtype=np.float32)  # [8, 8, 128, 512]
        out[c * MS:(c + 1) * MS] = blk.transpose(1, 2, 0, 3).reshape(MS, N)
    out *= inv_cols[None, :]
    return out
